# revision 1
# baseline (speedup 1.0000x reference)
"""Multi-head causal attention (B=2, S=2048, D=1024, H=16, Dh=64) on 8 TRN2
NeuronCores.

Sharding: core c = 4*b + g handles batch b (2-way data parallel) and head
group g (4-way tensor parallel over the 16 heads: heads 4g..4g+3, i.e. a
256-column slice of W_q/W_k/W_v, and the matching 256-row slice of W_o).
Each core returns a partial output [S, D]; the host sums the 4 partials per
batch and adds b_o (row-parallel out-projection reduce).

On-core layout is "K-major" flash attention: scores are computed transposed
(S^T[k, q] = K q^T) so softmax's sum over k can be folded into the
attn@V matmul by augmenting V's stationary tile with 64 columns of ones
(denominator lands in the other half of the PSUM partition range).
All matmuls run in float32r (full-rate fp32 on the PE array).
"""

import numpy as np
from contextlib import ExitStack

import concourse.bass as bass
import concourse.bacc as bacc
import concourse.tile as tile
import concourse.mybir as mybir
from concourse.bass_utils import run_bass_kernel_spmd

F32 = mybir.dt.float32
F32R = mybir.dt.float32r
AF = mybir.ActivationFunctionType

B = 2
S = 2048
D = 1024
DC = 256  # head dims per core (4 heads x 64)
N_CORES = 8
NT = D // 128  # 8 input-dim tiles
ST = S // 128  # 16 sequence tiles


def _slices512(off, end):
    """Bank-aligned column slices of [off, end): split at 512 boundaries."""
    out = []
    a = off
    while a < end:
        b = min(end, (a // 512 + 1) * 512)
        out.append((a, b))
        a = b
    return out


def _build():
    nc = bacc.Bacc("TRN2", target_bir_lowering=False, debug=False,
                   num_devices=N_CORES)
    xt = nc.dram_tensor("xt", [D, S], F32R, kind="ExternalInput").ap()
    wq = nc.dram_tensor("wq", [D, DC], F32R, kind="ExternalInput").ap()
    wk = nc.dram_tensor("wk", [D, DC], F32R, kind="ExternalInput").ap()
    wv = nc.dram_tensor("wv", [D, DC], F32R, kind="ExternalInput").ap()
    wo = nc.dram_tensor("wo", [DC, D], F32R, kind="ExternalInput").ap()
    mk = nc.dram_tensor("mk", [128, 128], F32, kind="ExternalInput").ap()
    y = nc.dram_tensor("y", [S, D], F32, kind="ExternalOutput").ap()

    with tile.TileContext(nc) as tc, ExitStack() as stk:
        persist = stk.enter_context(tc.tile_pool(name="persist", bufs=1))
        qt_sb = persist.tile([128, 2 * S], F32R)   # Q^T/8: dq-tile j at cols 2048j
        kt_sb = persist.tile([128, 2 * S], F32R)   # K^T
        # V per k-tile block of 512 cols: head h sub-block of 128 cols =
        # [V_h | ones] for even h, [ones | V_h] for odd h.
        v_sb = persist.tile([128, ST * 512], F32R)
        ct_sb = persist.tile([128, 2 * S], F32R)   # normalized ctx^T
        wo_sb = persist.tile([128, 2 * D], F32R)   # W_o slice: dc-tile d at cols 1024d
        mk_sb = persist.tile([128, 128], F32)      # mask[k, q] = (k <= q)

        nc.sync.dma_start(out=mk_sb[:], in_=mk[:, :])
        for d in range(2):
            nc.sync.dma_start(out=wo_sb[:, 1024 * d:1024 * (d + 1)],
                              in_=wo[128 * d:128 * (d + 1), :])

        # ---- projections: Q^T, K^T (dq on partitions) and V ----
        with tc.tile_pool(name="stg1", bufs=1) as stg1, \
             tc.tile_pool(name="ppq", bufs=2, space="PSUM") as ppq, \
             tc.tile_pool(name="ppv", bufs=4, space="PSUM") as ppv:
            xt_sb = stg1.tile([128, NT * S], F32R)
            wq_sb = stg1.tile([128, NT * DC], F32R)
            wk_sb = stg1.tile([128, NT * DC], F32R)
            wv_sb = stg1.tile([128, NT * DC], F32R)
            for i in range(NT):
                nc.sync.dma_start(out=xt_sb[:, S * i:S * (i + 1)],
                                  in_=xt[128 * i:128 * (i + 1), :])
                nc.sync.dma_start(out=wq_sb[:, DC * i:DC * (i + 1)],
                                  in_=wq[128 * i:128 * (i + 1), :])
                nc.sync.dma_start(out=wk_sb[:, DC * i:DC * (i + 1)],
                                  in_=wk[128 * i:128 * (i + 1), :])
                nc.sync.dma_start(out=wv_sb[:, DC * i:DC * (i + 1)],
                                  in_=wv[128 * i:128 * (i + 1), :])

            def qk_half(j, w_sb, dst, half):
                ps = ppq.tile([128, 1024], F32, tag="ppq", name=f"pq{j}{half}")
                for i in range(NT):
                    for a, b in ((0, 512), (512, 1024)):
                        nc.tensor.matmul(
                            ps[:, a:b],
                            lhsT=(w_sb[:, DC * i + 128 * j:
                                       DC * i + 128 * (j + 1)]),
                            rhs=(xt_sb[:, S * i + 1024 * half + a:
                                       S * i + 1024 * half + b]),
                            start=(i == 0), stop=(i == NT - 1))
                nc.scalar.copy(
                    dst[:, 2048 * j + 1024 * half:
                        2048 * j + 1024 * (half + 1)], ps[:, :])

            def v_round(st):
                nc.vector.memset(
                    v_sb[:, 512 * st:512 * (st + 1)].bitcast(F32), 1.0)
                pv = ppv.tile([128, 256], F32, tag="ppv", name=f"pv{st}")
                for i in range(NT):
                    nc.tensor.matmul(
                        pv[:, 0:256],
                        lhsT=(xt_sb[:, S * i + 128 * st:
                                    S * i + 128 * (st + 1)]),
                        rhs=(wv_sb[:, DC * i:DC * (i + 1)]),
                        start=(i == 0), stop=(i == NT - 1))
                base = 512 * st
                blk = v_sb[:, base:base + 512].rearrange(
                    "p (h c) -> p h c", c=256)
                srcv = pv[:, 0:256].rearrange("p (h c) -> p h c", c=128)
                nc.vector.tensor_copy(blk[:, :, 0:64], srcv[:, :, 0:64])
                nc.vector.tensor_copy(blk[:, :, 192:256], srcv[:, :, 64:128])

            for half in range(2):
                qk_half(0, wq_sb, qt_sb, half)
            for half in range(2):
                qk_half(0, wk_sb, kt_sb, half)
            for st in range(8):
                v_round(st)
            for half in range(2):
                qk_half(1, wq_sb, qt_sb, half)
            for half in range(2):
                qk_half(1, wk_sb, kt_sb, half)
            for st in range(8, ST):
                v_round(st)

        # ---- attention per (head, 1024-query-chunk) ----
        with tc.tile_pool(name="sp", bufs=2, space="PSUM") as sp, \
             tc.tile_pool(name="cp", bufs=2, space="PSUM") as cp, \
             tc.tile_pool(name="ep", bufs=10) as ep, \
             tc.tile_pool(name="rp", bufs=6) as rp:
            for h in range(4):
                jh = h // 2
                hb = 64 * (h % 2)   # partition base where ctx lands
                dr = 64 - hb        # partition base where denominator lands
                for qc in range(2):
                    ctx_ps = cp.tile([128, 1024], F32, tag="ctx",
                                     name=f"cx{h}{qc}")
                    kt_max = 8 * qc + 7

                    def ctx_round(kt, e_sb, off):
                        for a, b in _slices512(off, 1024):
                            last_kt = 8 * qc + (3 if b <= 512 else 7)
                            nc.tensor.matmul(
                                ctx_ps[:, a:b],
                                lhsT=(v_sb[:, 512 * kt + 128 * h:
                                           512 * kt + 128 * (h + 1)]),
                                rhs=(e_sb[:, a:b]),
                                start=(kt == 0), stop=(kt == last_kt))

                    for kt in range(kt_max + 1):
                        q_lo = max(1024 * qc, 128 * kt)
                        off = q_lo - 1024 * qc
                        s_ps = sp.tile([128, 1024], F32, tag="s",
                                       name=f"s{h}{qc}{kt}")
                        e_sb = ep.tile([128, 1024], F32R, tag="e",
                                       name=f"e{h}{qc}{kt}")
                        for a, b in _slices512(off, 1024):
                            # fp32r needs a >=256-wide moving operand for
                            # full rate; widen narrow leading slices downward
                            # (extra cols land before `off`, never read)
                            a = min(a, b - 256)
                            nc.tensor.matmul(
                                s_ps[:, a:b],
                                lhsT=(kt_sb[hb:hb + 64,
                                            2048 * jh + 128 * kt:
                                            2048 * jh + 128 * (kt + 1)]),
                                rhs=(qt_sb[hb:hb + 64,
                                           2048 * jh + 1024 * qc + a:
                                           2048 * jh + 1024 * qc + b]),
                                start=True, stop=True)
                        nc.scalar.activation(e_sb[:, off:1024],
                                             s_ps[:, off:1024], AF.Exp)
                        if 128 * kt >= 1024 * qc:
                            # diagonal block: zero strictly-lower (k > q)
                            nc.gpsimd.tensor_mul(e_sb[:, off:off + 128],
                                                 e_sb[:, off:off + 128],
                                                 mk_sb[:, :])
                        ctx_round(kt, e_sb, off)

                    # normalize: ctx rows are [hb, hb+64), denominator rows
                    # (sum of exp) are [dr, dr+64), replicated columns.
                    rcp = rp.tile([128, 1024], F32, tag="rcp", name=f"r{h}{qc}")
                    rcb = rp.tile([128, 1024], F32, tag="rcb", name=f"rb{h}{qc}")
                    # NB: reciprocal_approx_* miscompute at partition base != 0
                    nc.vector.reciprocal(rcp[dr:dr + 1, :], ctx_ps[dr:dr + 1, :])
                    if dr == 0:
                        # gpsimd broadcast (reads true partition 0 only)
                        nc.gpsimd.partition_broadcast(rcb[:, :], rcp[0:1, :])
                    else:
                        nc.sync.dma_start(
                            out=rcb[hb:hb + 64, :],
                            in_=rcp[dr:dr + 1, :].unsqueeze(1)
                            .to_broadcast((1, 64, 1024)))
                    nc.vector.tensor_mul(
                        ct_sb[hb:hb + 64,
                              2048 * jh + 1024 * qc:
                              2048 * jh + 1024 * (qc + 1)],
                        ctx_ps[hb:hb + 64, :], rcb[hb:hb + 64, :])

        # ---- partial out-projection y = ctx @ W_o[slice] ----
        with tc.tile_pool(name="op", bufs=2, space="PSUM") as op, \
             tc.tile_pool(name="ob", bufs=6) as ob:
            for st in range(ST):
                o_ps = op.tile([128, 1024], F32, tag="o", name=f"op{st}")
                for d in range(2):
                    for a, b in ((0, 512), (512, 1024)):
                        nc.tensor.matmul(
                            o_ps[:, a:b],
                            lhsT=(ct_sb[:, 2048 * d + 128 * st:
                                        2048 * d + 128 * (st + 1)]),
                            rhs=(wo_sb[:, 1024 * d + a:1024 * d + b]),
                            start=(d == 0), stop=(d == 1))
                o_sb = ob.tile([128, 1024], F32, tag="osb", name=f"ob{st}")
                nc.vector.tensor_copy(o_sb[:, :], o_ps[:, :])
                nc.sync.dma_start(out=y[128 * st:128 * (st + 1), :],
                                  in_=o_sb[:, :])

    nc.compile()
    return nc


_nc = None


def kernel(x, W_q, W_k, W_v, W_o, b_o):
    global _nc
    x = np.ascontiguousarray(np.asarray(x, dtype=np.float32))
    W_q = np.asarray(W_q, dtype=np.float32)
    W_k = np.asarray(W_k, dtype=np.float32)
    W_v = np.asarray(W_v, dtype=np.float32)
    W_o = np.asarray(W_o, dtype=np.float32)
    b_o = np.asarray(b_o, dtype=np.float32)

    if _nc is None:
        _nc = _build()

    mask = np.triu(np.ones((128, 128), dtype=np.float32))  # 1 where k <= q
    in_maps = []
    for c in range(N_CORES):
        b = c // 4
        g = c % 4
        sl = slice(DC * g, DC * (g + 1))
        in_maps.append({
            "xt": np.ascontiguousarray(x[b].T),
            "wq": np.ascontiguousarray(W_q[:, sl]) * 0.125,  # fold 1/sqrt(Dh)
            "wk": np.ascontiguousarray(W_k[:, sl]),
            "wv": np.ascontiguousarray(W_v[:, sl]),
            "wo": np.ascontiguousarray(W_o[sl, :]),
            "mk": mask,
        })

    res = run_bass_kernel_spmd(_nc, in_maps, list(range(N_CORES)))
    parts = [res.results[c]["y"] for c in range(N_CORES)]
    out = np.empty((B, S, D), dtype=np.float32)
    for b in range(B):
        acc = np.zeros((S, D), dtype=np.float64)
        for g in range(4):
            acc += parts[4 * b + g]
        acc += b_o
        out[b] = acc.astype(np.float32)
    return out

